# revision 6
# baseline (speedup 1.0000x reference)
"""Trainium2 Bass kernel: 2-layer BiLSTM classifier (B=32, I=128, T=512, H=512, O=10).

Sharding: 8 cores = 4 batch groups x 2 directions. Each core runs both layers
for ONE direction on 8 batch rows; bwd cores receive time-reversed input so
the on-device program is direction-agnostic. Between layers, core pairs
AllGather their layer-0 hidden states (h1^T, chunk-major) for the layer-1
input projection; the final linear is summed across the pair via AllReduce.

Per layer on each core:
  xp = Wih^T @ input + b   (bulk projection -> DRAM, streamed back per 8 steps)
  per step t: g[b,4H] = xp_t + Whh^T h_{t-1}  (PSUM, 4 gate banks i,f,g,o)
    i,f,o = sigmoid(bank), g~ = tanh(bank)    (per-bank acts, overlap MMs)
    c = f*c + i*g~; h = o*tanh(c)             (DVE + Pool)
    h^T via PE transpose -> ring -> (L0) DRAM flush per 8 steps
Gate-bank emission order g,i,f,o so the c-chain starts early.
"""

import numpy as np

B_FULL, I_IN, T, H, O = 32, 128, 512, 512, 10
NCORES = 8
B = 8                      # batch rows per core
G4 = 4 * H                 # 2048
NB = 4                     # gate banks (tile index): 0=i 1=f 2=g 3=o
UNROLL = 8
NBLK = T // UNROLL         # 64
TB = T * B                 # 4096
MT = TB // 128             # 32 m-tiles for projections
BORDER = (2, 0, 1, 3)      # bank emission order: g, i, f, o
TRB = 2                    # transposes land in bank BORDER[0]'s region

_CACHE = {}


def _build_nc(sim=False, ccq=False, ndum=0, ndum_blk=0, skip=()):
    # skip: timing probes - e.g. ("rec1",) or ("rec0", "rec1") (wrong results)
    # ccq: timing probe - quarter-size AllGather (wrong results)
    # ndum / ndum_blk: keep-warm dummy matmuls per step / at block end
    # sim=True: replace collectives with local DMA stand-ins so the
    # single-core TimelineSim can run the program (analysis only)
    import contextlib

    import concourse.bass as bass
    import concourse.mybir as mybir
    import concourse.tile as tile
    from concourse import bacc
    from concourse.bass import ds

    F32 = mybir.dt.float32
    F32R = mybir.dt.float32r
    AF = mybir.ActivationFunctionType
    OP = mybir.AluOpType

    nc = bacc.Bacc("TRN2", target_bir_lowering=False, debug=False, num_devices=NCORES)

    # ---------------- I/O ----------------
    xT_d = nc.dram_tensor("xT", [I_IN, TB], F32R, kind="ExternalInput")
    wih0_d = nc.dram_tensor("wih0", [I_IN, G4], F32R, kind="ExternalInput")
    F8 = mybir.dt.float8e4
    whh0_d = nc.dram_tensor("whh0", [128, 4 * G4], F8, kind="ExternalInput")
    whh1_d = nc.dram_tensor("whh1", [128, 4 * G4], F8, kind="ExternalInput")
    wih1o_d = nc.dram_tensor("wih1o", [128, 4 * G4], mybir.dt.bfloat16, kind="ExternalInput")
    wih1x_d = nc.dram_tensor("wih1x", [128, 4 * G4], mybir.dt.bfloat16, kind="ExternalInput")
    b0_d = nc.dram_tensor("b0", [1, G4], F32R, kind="ExternalInput")
    b1_d = nc.dram_tensor("b1", [1, G4], F32R, kind="ExternalInput")
    wlin_d = nc.dram_tensor("wlin", [128, 4 * O], F32R, kind="ExternalInput")
    blin_d = nc.dram_tensor("blin", [1, O], F32R, kind="ExternalInput")
    ones_d = nc.dram_tensor("ones", [1, 128], F32R, kind="ExternalInput")
    i64_d = nc.dram_tensor("i64", [64, 64], F32R, kind="ExternalInput")
    i8_d = nc.dram_tensor("i8", [B, B], F32R, kind="ExternalInput")
    out_d = nc.dram_tensor("out", [O, B], F32, kind="ExternalOutput")

    BF16 = mybir.dt.bfloat16
    QT = TB // 4   # time-cols per exchange quarter

    # ---------------- DRAM scratch ----------------
    xp_dram = nc.dram_tensor("xp", [TB, G4], F32R)         # shared by both layers
    # own h1^T in bf16, quarter-contiguous so each quarter AllGathers as soon
    # as its part of the layer-0 recurrence finishes (overlaps the rest)
    hs_q = [nc.dram_tensor(f"hs{q}", [4, 128, QT], BF16) for q in range(4)]
    ag_q = [nc.dram_tensor(f"ag{q}", [2 * 4 * 128, QT], BF16) for q in range(4)]
    fin_in = nc.dram_tensor("fin_in", [O, B], F32)
    fin_out = nc.dram_tensor("fin_out", [O, B], F32)
    groups = [[2 * p, 2 * p + 1] for p in range(4)]

    with tile.TileContext(nc) as tc:
        ctx = contextlib.ExitStack()
        sbuf = ctx.enter_context(tc.tile_pool(name="sbuf", bufs=1))
        psum = ctx.enter_context(tc.tile_pool(name="psum", bufs=1, space="PSUM"))
        xpp = ctx.enter_context(tc.tile_pool(name="xpp", bufs=3))
        gat = ctx.enter_context(tc.tile_pool(name="gat", bufs=2))
        smal = ctx.enter_context(tc.tile_pool(name="smal", bufs=2))
        prjp = ctx.enter_context(tc.tile_pool(name="prj", bufs=2))

        with ctx:
            # ---------- static tiles ----------
            ones_t = sbuf.tile([1, 128], F32R)
            nc.sync.dma_start(out=ones_t, in_=ones_d.ap())
            i64_t = sbuf.tile([64, 64], F32R)
            nc.sync.dma_start(out=i64_t, in_=i64_d.ap())
            i8_t = sbuf.tile([B, B], F32R)
            nc.sync.dma_start(out=i8_t, in_=i8_d.ap())
            blin_t = sbuf.tile([1, O], F32R)
            nc.sync.dma_start(out=blin_t, in_=blin_d.ap())
            wlin_t = sbuf.tile([128, 4 * O], F32R)
            nc.sync.dma_start(out=wlin_t, in_=wlin_d.ap())
            xT_t = sbuf.tile([I_IN, TB], F32R)
            nc.sync.dma_start(out=xT_t, in_=xT_d.ap())
            wih0_t = sbuf.tile([I_IN, G4], F32R)
            nc.sync.dma_start(out=wih0_t, in_=wih0_d.ap())
            b_t = {}
            for lb, src in ((0, b0_d), (1, b1_d)):
                b_t[lb] = sbuf.tile([1, G4], F32R, name=f"b{lb}")
                nc.sync.dma_start(out=b_t[lb], in_=src.ap())
            # big weight buffers: wbig (f32r) for whh0/whh1,
            # wb16 (bf16) for the wih1 halves
            wbig = sbuf.tile([128, 4 * G4], F8, name="wbig")
            nc.gpsimd.dma_start(out=wbig, in_=whh0_d.ap())
            wb16 = sbuf.tile([128, 4 * G4], BF16, name="wb16")
            wb16x = sbuf.tile([128, 4 * G4], BF16, name="wb16x")

            ring = sbuf.tile([128, UNROLL * 8 * B], F8, name="ring")
            MPD = mybir.MatmulPerfMode.DoubleRow
            pooled_bh = sbuf.tile([B, H], F32, name="pooled_bh")
            ring2 = sbuf.tile([128, UNROLL * 4 * B], BF16, name="ring2")
            h_st = [sbuf.tile([B, H], F32R, name="h_even"),
                    sbuf.tile([B, H], F32R, name="h_odd")]
            c_t = sbuf.tile([B, H], F32, name="c")

            st = {}  # per-step state passed between emission segments
            gps = {}  # persistent psum group tiles, keyed by step parity

            def emit_whh_pair(Gt, bank, prev, p, stop):
                # fp8 DoubleRow: one MM contracts two 128-deep h chunks
                lhsT = bass.AP(
                    tensor=ring.tensor,
                    offset=ring[:, prev * 64 + 32 * p : prev * 64 + 32 * p + 8].offset,
                    ap=[list(ring.ap[0]), [16, 2], [1, 8]])
                rhs = bass.AP(
                    tensor=wbig.tensor,
                    offset=wbig[:, 2 * p * G4 + 512 * bank : 2 * p * G4 + 512 * bank + 512].offset,
                    ap=[list(wbig.ap[0]), [G4, 2], [1, 512]])
                nc.tensor.matmul(
                    Gt[bank][0:B, :], lhsT=lhsT, rhs=rhs,
                    start=False, stop=stop, perf_mode=MPD)

            def emit_cc(q):
                if sim:
                    aqo = ag_q[q].ap().rearrange("(r x) t -> r x t", r=2)
                    flat = hs_q[q].ap().rearrange("k p t -> (k p) t")
                    nc.gpsimd.dma_start(out=aqo[0], in_=flat)
                    nc.gpsimd.dma_start(out=aqo[1], in_=flat)
                else:
                    nc.gpsimd.collective_compute(
                        "AllGather", mybir.AluOpType.bypass,
                        replica_groups=groups,
                        ins=[hs_q[q].ap()], outs=[ag_q[q].ap()])

            # ================= recurrence =================
            def emit_mms(layer, u, first, xp_blk):
                Gt = gps[u % 2]
                prev = (u - 1) % UNROLL
                for bank in BORDER[:3]:
                    nc.tensor.matmul(
                        Gt[bank][0:B, :],
                        lhsT=i64_t[:, 8 * u : 8 * u + 8],
                        rhs=xp_blk[:, 512 * bank : 512 * (bank + 1)],
                        start=True, stop=first)
                nc.tensor.matmul(
                    Gt[3][0:B, :],
                    lhsT=i64_t[:, 8 * u : 8 * u + 8],
                    rhs=xp_blk[:, 512 * 3 : 512 * 4],
                    start=True, stop=first)
                # keep-warm dummies: PE work with no dependencies that fills
                # the tail-latency gap so the clock never throttles down;
                # they write the previous step's already-consumed g-bank
                # (the real transposes then overwrite cols 0:32)
                if "G" in st and ndum:
                    gprev = st["G"][TRB]
                    for _ in range(ndum):
                        nc.tensor.matmul(
                            gprev[0:B, :], lhsT=i64_t[:, 0:8],
                            rhs=xp_blk[:, 0:512], start=True, stop=True)
                # previous step's transposes + ring copies, interleaved with
                # the first (g-bank) whh MMs: each pair of ring chunks enables
                # the whh MMs for those k-chunks, so the PE never head-blocks
                # on the second transpose pair
                pt = st.pop("h", None)
                if pt is not None:
                    p_h, p_G, p_u = pt, st.pop("G"), st.pop("u")
                    p_trv = p_G[TRB].bitcast(F32R)
                    p_slot = p_u % UNROLL
                acts = {}
                tmp = smal.tile([B, H], F32, tag="tmp", name="tmp")
                for bank in BORDER[:3]:
                    if bank == 2 and pt is not None:
                        if layer == 1 and "pool" not in skip:
                            # previous step's h is ready: accumulate it now so
                            # the Pool queue never blocks this step's chain
                            for half in (0, 1):
                                sl = slice(256 * half, 256 * (half + 1))
                                nc.gpsimd.tensor_tensor(
                                    out=pooled_bh[:, sl], in0=pooled_bh[:, sl],
                                    in1=p_h[:, sl], op=OP.add)
                        for half in (0, 1):
                            for k in (2 * half, 2 * half + 1):
                                nc.tensor.transpose(
                                    p_trv[:, 8 * k : 8 * k + 8],
                                    p_h[:, 128 * k : 128 * (k + 1)], i8_t)
                            rdst = bass.AP(
                                tensor=ring.tensor,
                                offset=ring[:, p_slot * 64 + 32 * half : p_slot * 64 + 32 * half + 8].offset,
                                ap=[list(ring.ap[0]), [16, 2], [1, 8]])
                            nc.vector.tensor_copy(
                                rdst,
                                p_trv[:, 16 * half : 16 * half + 16]
                                .rearrange("p (a b) -> p a b", a=2))
                            if layer == 0:
                                nc.vector.tensor_copy(
                                    ring2[:, p_slot * 32 + 16 * half : p_slot * 32 + 16 * half + 16],
                                    p_trv[:, 16 * half : 16 * half + 16])
                            if not first:
                                emit_whh_pair(Gt, 2, prev, half, stop=(half == 1))

                    elif bank == 2 and not first:
                        for p in (0, 1):
                            emit_whh_pair(Gt, 2, prev, p, stop=(p == 1))
                    if bank != 2 and not first:
                        for p in (0, 1):
                            emit_whh_pair(Gt, bank, prev, p, stop=(p == 1))
                    t_ = gat.tile([B, 512], F32, tag=f"t{bank}", name=f"t{bank}")
                    nc.scalar.activation(
                        t_, Gt[bank][0:B, :], AF.Tanh if bank == 2 else AF.Sigmoid)
                    acts[bank] = t_
                    if bank == 0:
                        # i*g~ on Pool in halves, early and off the chain
                        for hh in (0, 1):
                            sl = slice(256 * hh, 256 * (hh + 1))
                            nc.gpsimd.tensor_tensor(
                                out=tmp[:, sl], in0=acts[0][:, sl],
                                in1=acts[2][:, sl], op=OP.mult)
                # o-gate in halves: each half's whh MMs stop early so the
                # o-act and the h chain start before the full bank is done
                to = gat.tile([B, 512], F32, tag="t3", name="t3")
                cf = smal.tile([B, H], F32, tag="cf", name="cf")
                tch = smal.tile([B, H], F32, tag="tch", name="tch")
                h_t = h_st[u % 2]
                if not first:
                    for p in (0, 1):
                        emit_whh_pair(Gt, 3, prev, p, stop=(p == 1))
                for hh in (0, 1):
                    sl = slice(256 * hh, 256 * (hh + 1))
                    nc.vector.tensor_tensor(
                        out=cf[:, sl], in0=acts[1][:, sl], in1=c_t[:, sl], op=OP.mult)
                    nc.vector.tensor_tensor(
                        out=c_t[:, sl], in0=cf[:, sl], in1=tmp[:, sl], op=OP.add)
                    nc.scalar.activation(to[:, sl], Gt[3][0:B, sl], AF.Sigmoid)
                    nc.scalar.activation(tch[:, sl], c_t[:, sl], AF.Tanh)
                    nc.vector.tensor_tensor(
                        out=h_t[:, sl], in0=to[:, sl], in1=tch[:, sl], op=OP.mult)
                st["h"] = h_t
                st["G"] = Gt
                st["u"] = u

            def emit_prev_tail(layer):
                if "h" not in st:
                    return
                h_t, Gt, u = st.pop("h"), st.pop("G"), st.pop("u")
                if layer == 1 and "pool" not in skip:
                    nc.gpsimd.tensor_tensor(
                        out=pooled_bh, in0=pooled_bh, in1=h_t, op=OP.add)
                trv = Gt[TRB].bitcast(F32R)
                slot = u % UNROLL
                for k in range(4):
                    nc.tensor.transpose(
                        trv[:, 8 * k : 8 * k + 8],
                        h_t[:, 128 * k : 128 * (k + 1)], i8_t)
                    if k % 2 == 1:
                        half = k // 2
                        rdst = bass.AP(
                            tensor=ring.tensor,
                            offset=ring[:, slot * 64 + 32 * half : slot * 64 + 32 * half + 8].offset,
                            ap=[list(ring.ap[0]), [16, 2], [1, 8]])
                        nc.vector.tensor_copy(
                            rdst,
                            trv[:, 16 * half : 16 * half + 16]
                            .rearrange("p (a b) -> p a b", a=2))
                if layer == 0:
                    nc.vector.tensor_copy(
                        ring2[:, slot * 32 : (slot + 1) * 32], trv[:, 0:32])


            def emit_flush(q, i):
                # block i's ring2 slots -> hs_q[q]; i is the block-local
                # offset within the quarter (static or loop reg)
                dstall = hs_q[q].ap().rearrange("k p t -> p k t")
                for k in range(4):
                    src = bass.AP(
                        tensor=ring2.tensor,
                        offset=ring2[:, 8 * k : 8 * k + 8].offset,
                        ap=[list(ring2.ap[0]), [32, UNROLL], [1, 8]])
                    dst = (dstall[:, k, 64 * i : 64 * (i + 1)] if isinstance(i, int)
                           else dstall[:, k, ds(i * 64, 64)])
                    nc.scalar.dma_start(out=dst, in_=src)

            def recurrence(layer):
                nc.vector.memset(c_t, 0.0)
                if layer == 1:
                    nc.gpsimd.memset(pooled_bh, 0.0)
                st.clear()
                gps[0] = [psum.tile([128, 512], F32, tag=f"gA{b}", name=f"gA{b}")
                          for b in range(NB)]
                gps[1] = [psum.tile([128, 512], F32, tag=f"gB{b}", name=f"gB{b}")
                          for b in range(NB)]

                def block(q, i, first_block, flush_prev):
                    # i = global block index (static int or loop reg);
                    # tails stay deferred across blocks (all handoff tiles are
                    # static), so each block's slot-7 copy lands in the next
                    # block's first segment -- flush the PREVIOUS block there
                    xp_blk = xpp.tile([64, G4], F32R, tag="xpb", name="xpb")
                    if first_block:
                        nc.sync.dma_start(out=xp_blk, in_=xp_dram.ap()[0:64, :])
                    else:
                        nc.sync.dma_start(
                            out=xp_blk, in_=xp_dram.ap()[ds(i * 64, 64), :])
                    emit_mms(layer, 0, first_block and q == 0, xp_blk)
                    if layer == 0 and flush_prev is not None:
                        emit_flush(*flush_prev)
                        if flush_prev[0] != q:
                            emit_cc(flush_prev[0])
                    for u in range(1, UNROLL):
                        emit_mms(layer, u, False, xp_blk)

                if layer == 0:
                    for q in range(4):
                        # python-emitted first block of the quarter: flushes
                        # the previous quarter's last block, then launches
                        # that quarter's AllGather (overlaps this quarter)
                        fp = None if q == 0 else (q - 1, 15)
                        block(q, 16 * q, q == 0, fp)
                        with tc.For_i(16 * q + 1, 16 * q + 16) as i:
                            block(q, i, False, (q, i - 16 * q - 1))
                    emit_prev_tail(0)
                    emit_flush(3, 15)
                    emit_cc(3)
                else:
                    block(0, 0, True, None)
                    with tc.For_i(1, NBLK) as i:
                        block(0, i, False, None)
                    emit_prev_tail(1)

            # ================= projections =================
            def proj_psum():
                return [[psum.tile([128, 512], F32, tag=f"g{pc}{b}", name=f"pp{pc}{b}")
                         for b in range(NB)] for pc in "AB"]

            def proj0():
                ppg = proj_psum()
                for m in range(MT):
                    pp = ppg[m % 2]
                    ev = prjp.tile([128, G4], F32R, tag="ev", name="ev")
                    for bank in range(NB):
                        nc.tensor.matmul(
                            pp[bank], lhsT=ones_t[0:1, 0:128],
                            rhs=b_t[0][:, 512 * bank : 512 * (bank + 1)],
                            start=True, stop=False)
                        nc.tensor.matmul(
                            pp[bank],
                            lhsT=xT_t[:, 128 * m : 128 * (m + 1)],
                            rhs=wih0_t[:, 512 * bank : 512 * (bank + 1)],
                            start=False, stop=True)
                        nc.scalar.activation(
                            ev[:, 512 * bank : 512 * (bank + 1)], pp[bank], AF.Identity)
                    nc.gpsimd.dma_start(
                        out=xp_dram.ap()[128 * m : 128 * (m + 1), :], in_=ev)

            def proj1():
                # single pass: own-half and partner-half contributions
                # accumulate into the same PSUM group, so xp is written to
                # DRAM once (no read-back / add pass, no weight swap)
                pid = nc.sync.partition_id()
                roff = (1 - pid % 2) * 512
                ppg = proj_psum()
                for m in range(MT):
                    q, lm = m // 8, m % 8
                    mx = MT - 1 - m               # partner tile index
                    qx, lx = mx // 8, mx % 8
                    hto = prjp.tile([128, 512], BF16, tag="hto", name="hto")
                    nc.sync.dma_start(
                        out=hto.rearrange("p (k t) -> p k t", k=4),
                        in_=hs_q[q].ap().rearrange("k p t -> p k t")
                        [:, :, 128 * lm : 128 * (lm + 1)])
                    htx = prjp.tile([128, 512], BF16, tag="htx", name="htx")
                    for k in range(4):
                        nc.sync.dma_start(
                            out=htx[:, 128 * k : 128 * (k + 1)],
                            in_=ag_q[qx].ap()[ds(roff + 128 * k, 128),
                                              128 * lx : 128 * (lx + 1)])
                    # reverse the partner tile into my time order (DVE copy,
                    # matmul operand APs must stay 2D)
                    hto2 = prjp.tile([128, 512], BF16, tag="hto2", name="hto2")
                    rev = bass.AP(
                        tensor=htx.tensor,
                        offset=htx[:, 120:128].offset,
                        ap=[list(htx.ap[0]), [128, 4], [-8, 16], [1, 8]])
                    nc.vector.tensor_copy(
                        hto2.rearrange("p (a b c) -> p a b c", a=4, b=16, c=8), rev)
                    pp = ppg[m % 2]
                    ev = prjp.tile([128, G4], F32R, tag="ev", name="ev")
                    for bank in range(NB):
                        nc.tensor.matmul(
                            pp[bank], lhsT=ones_t[0:1, 0:128],
                            rhs=b_t[1][:, 512 * bank : 512 * (bank + 1)],
                            start=True, stop=False)
                        for k in range(4):
                            nc.tensor.matmul(
                                pp[bank],
                                lhsT=hto[:, 128 * k : 128 * (k + 1)],
                                rhs=wb16[:, k * G4 + 512 * bank : k * G4 + 512 * bank + 512],
                                start=False, stop=False)
                        for k in range(4):
                            nc.tensor.matmul(
                                pp[bank],
                                lhsT=hto2[:, 128 * k : 128 * (k + 1)],
                                rhs=wb16x[:, k * G4 + 512 * bank : k * G4 + 512 * bank + 512],
                                start=False, stop=(k == 3))
                        nc.scalar.activation(
                            ev[:, 512 * bank : 512 * (bank + 1)], pp[bank], AF.Identity)
                    nc.gpsimd.dma_start(
                        out=xp_dram.ap()[128 * m : 128 * (m + 1), :], in_=ev)

            # ================= run =================
            proj0()
            nc.gpsimd.dma_start(out=wb16, in_=wih1o_d.ap())
            nc.gpsimd.dma_start(out=wb16x, in_=wih1x_d.ap())
            if "rec0" not in skip:
                recurrence(0)
            else:
                for q in range(4):
                    emit_cc(q)
            proj1()
            nc.gpsimd.dma_start(out=wbig, in_=whh1_d.ap())
            if "rec1" not in skip:
                recurrence(1)
            else:
                nc.gpsimd.memset(pooled_bh, 0.0)

            # ---------- final linear ----------
            ftr = gps[0][TRB].bitcast(F32R)
            pbr = sbuf.tile([B, H], F32R, name="pbr")
            nc.vector.tensor_copy(pbr, pooled_bh)
            for k in range(4):
                nc.tensor.transpose(
                    ftr[:, 8 * k : 8 * k + 8], pbr[:, 128 * k : 128 * (k + 1)], i8_t)
            plr = sbuf.tile([128, 4 * B], F32R, name="plr")
            nc.vector.tensor_copy(plr, ftr[:, 0:32])
            fin_full = psum.tile([128, 512], F32, tag="gA0", name="fin_full")
            fin_ps = fin_full[0:O, 0:B]
            nc.tensor.matmul(fin_ps, lhsT=blin_t, rhs=ones_t[0:1, 0:B],
                             start=True, stop=False)
            for k in range(4):
                nc.tensor.matmul(
                    fin_ps, lhsT=wlin_t[:, O * k : O * (k + 1)],
                    rhs=plr[:, B * k : B * (k + 1)],
                    start=False, stop=(k == 3))
            fin_sb = sbuf.tile([O, B], F32, name="fin_sb")
            nc.scalar.copy(fin_sb, fin_ps)
            nc.sync.dma_start(out=fin_in.ap(), in_=fin_sb)
            if sim:
                nc.gpsimd.dma_start(out=fin_out.ap(), in_=fin_in.ap())
            else:
                nc.gpsimd.collective_compute(
                    "AllReduce", mybir.AluOpType.add, replica_groups=groups,
                    ins=[fin_in.ap()], outs=[fin_out.ap()])
            fin2 = sbuf.tile([O, B], F32, name="fin2")
            nc.sync.dma_start(out=fin2, in_=fin_out.ap())
            nc.sync.dma_start(out=out_d.ap(), in_=fin2)

    nc.compile()
    return nc


# ======================= host side =======================

def _chunked(W):
    # W: [4H, K] -> [128, (K/128)*4H] chunk-major along K
    f32 = np.float32
    K = W.shape[1]
    return np.ascontiguousarray(
        W.astype(f32).T.reshape(K // 128, 128, G4).transpose(1, 0, 2)
        .reshape(128, (K // 128) * G4))


def _make_in_maps(inputs):
    import ml_dtypes
    f32 = np.float32
    x = np.asarray(inputs["x"], f32)
    maps = []
    for c in range(NCORES):
        p, r = c // 2, c % 2
        d = "f" if r == 0 else "r"
        xs = x[B * p : B * (p + 1)]                  # [8, 128, 512]
        arr = xs.transpose(1, 2, 0)                  # [128, T, 8]
        if r == 1:
            arr = arr[:, ::-1, :]
        m = {"xT": np.ascontiguousarray(arr.reshape(I_IN, TB))}
        m["wih0"] = np.ascontiguousarray(inputs[f"Wih0{d}"].astype(f32).T)
        f8 = ml_dtypes.float8_e4m3fn
        m["whh0"] = _chunked(inputs[f"Whh0{d}"]).astype(f8)
        m["whh1"] = _chunked(inputs[f"Whh1{d}"]).astype(f8)
        m["b0"] = inputs[f"b0{d}"].astype(f32)[None, :]
        m["b1"] = inputs[f"b1{d}"].astype(f32)[None, :]
        bf16 = ml_dtypes.bfloat16
        W1 = inputs[f"Wih1{d}"].astype(f32)          # [2048, 1024]
        m["wih1o"] = _chunked(W1[:, 512 * r : 512 * (r + 1)]).astype(bf16)
        m["wih1x"] = _chunked(W1[:, 512 * (1 - r) : 512 * (2 - r)]).astype(bf16)
        wl = (inputs["Wlin"].astype(f32)[:, 512 * r : 512 * (r + 1)] / T)
        m["wlin"] = np.ascontiguousarray(
            wl.T.reshape(4, 128, O).transpose(1, 0, 2).reshape(128, 4 * O))
        m["blin"] = (inputs["blin"].astype(f32) * 0.5)[None, :]
        m["ones"] = np.ones((1, 128), f32)
        m["i64"] = np.eye(64, dtype=f32)
        m["i8"] = np.eye(B, dtype=f32)
        maps.append(m)
    return maps


def _assemble_out(results):
    out = np.zeros((B_FULL, O), np.float32)
    for p in range(4):
        out[B * p : B * (p + 1)] = results[2 * p]["out"].T
    return out


def kernel(**inputs):
    from concourse.bass_utils import run_bass_kernel_spmd

    if "nc" not in _CACHE:
        _CACHE["nc"] = _build_nc()
    nc = _CACHE["nc"]

    in_maps = _make_in_maps(inputs)
    res = run_bass_kernel_spmd(nc, in_maps, core_ids=list(range(NCORES)))
    return _assemble_out(res.results)

